# revision 10
# baseline (speedup 1.0000x reference)
# BiLSTM (2-layer, H=64, IN=1) + FC-on-last-timestep Trainium2 Bass kernel.
#
# Math shortcut: the final output only needs h1 = concat(fwd1[T-1], bwd1[T-1]).
#   - bwd1[T-1] is the FIRST step of layer-1's reverse scan -> one LSTM step.
#   - fwd1[T-1] needs the full layer-1 forward recurrence, which consumes
#     layer-0's fwd/bwd outputs at every t.
# So the kernel runs 3 full recurrences (bwd0, then fwd0+fwd1 in lockstep)
# plus one extra step and a tiny FC.
#
# Layout: features on partitions, batch on the free dim. B=512 is sharded
# 64 per core across 8 cores (pure data parallelism; weights replicated).
#
# All tanh's are computed via sigmoid (one ACT table): tanh(x) = 2*sigmoid(2x)-1.
# The kernel stores h_hat = h/2 everywhere and compensates by scaling every
# weight column that multiplies an h by 2 on the host. Gate pre-activations
# for the g-gate are doubled in the weights so sigmoid(2g) is produced
# directly. Biases ride the matmuls via ones-rows in the state tiles.
import numpy as np
from contextlib import ExitStack

import concourse.bass as bass
import concourse.tile as tile
from concourse import bacc
from concourse import mybir
from concourse.bass_utils import run_bass_kernel_spmd

F32 = mybir.dt.float32
SIG = mybir.ActivationFunctionType.Sigmoid
IDENT = mybir.ActivationFunctionType.Identity
MULT = mybir.AluOpType.mult
ADD = mybir.AluOpType.add
SUB = mybir.AluOpType.subtract

H = 64
NCORES = 8


def build_nc(T=512, BL=64):
    """Build the Bass program for one core handling a batch shard of BL."""
    nc = bacc.Bacc("TRN2", debug=False)

    # xs: bwd-phase x slot row, slot j holds x[j-1] (slot 0 zero).
    # xf: fwd-phase x row, t-major.
    xs = nc.dram_tensor("xs", [1, (T + 1) * BL], F32, kind="ExternalInput").ap()
    xf = nc.dram_tensor("xf", [4, (T // 4) * BL], F32, kind="ExternalInput").ap()
    l0f = nc.dram_tensor("l0f", [65, 256], F32, kind="ExternalInput").ap()
    l0r = nc.dram_tensor("l0r", [66, 256], F32, kind="ExternalInput").ap()
    xw = nc.dram_tensor("xw", [4, 256], F32, kind="ExternalInput").ap()
    l1 = nc.dram_tensor("l1", [65, 256], F32, kind="ExternalInput").ap()
    w1 = nc.dram_tensor("w1", [64, 512], F32, kind="ExternalInput").ap()
    w1r = nc.dram_tensor("w1r", [64, 512], F32, kind="ExternalInput").ap()
    b1r = nc.dram_tensor("b1r", [1, 256], F32, kind="ExternalInput").ap()
    fcw = nc.dram_tensor("fcw", [64, 2], F32, kind="ExternalInput").ap()
    fcb = nc.dram_tensor("fcb", [1, 1], F32, kind="ExternalInput").ap()
    out = nc.dram_tensor("out", [BL, 1], F32, kind="ExternalOutput").ap()

    with tile.TileContext(nc) as tc, ExitStack() as ctx:
        const = ctx.enter_context(tc.tile_pool(name="const", bufs=1))

        def T_(shape, name):
            return const.tile(shape, F32, name=name, tag=name)

        # ---------- persistent tiles ----------
        # bwd0 state+storage. Slot j (cols j*BL:(j+1)*BL), j in 0..T:
        #   rows 0:64 = hb_hat time-aligned at index j (written by scan step
        #   s=T-1-j), row 64 = x for the scan step reading this slot (= x[j-1],
        #   prefilled), row 65 = ones (bias row).
        HB = T_([66, (T + 1) * BL], "HBbuf")
        # x row for fwd x-matmuls, spread over the 4 legal AP start
        # partitions (0/32/64/96): row 32*(t%4), cols (t//4)*BL.
        xq = T_([97, (T // 4) * BL], "xqrow")
        l0f_t = T_([65, 256], "l0f_t")
        l0r_t = T_([66, 256], "l0r_t")
        xw_t = T_([97, 256], "xw_t")  # x-weight row at partitions 0/32/64/96
        l1_t = T_([65, 256], "l1_t")
        w1_t = T_([64, 512], "w1_t")
        w1r_t = T_([64, 512], "w1r_t")
        b1r_t = T_([1, 256], "b1r_t")
        fcw_t = T_([64, 2], "fcw_t")
        fcb_t = T_([1, 1], "fcb_t")
        ones_t = T_([1, BL], "ones_t")

        # phase A c-state ping-pong
        cA = [T_([64, BL], f"cA{i}") for i in range(2)]
        # phase B state: cols 0:BL = fwd0, cols BL:2BL = layer1.
        # rows 0:64 = h_hat, row 64 = ones (bias row).
        sB = [T_([65, 2 * BL], f"sB{i}") for i in range(2)]
        cB = [T_([64, 2 * BL], f"cB{i}") for i in range(2)]
        h1r_t = T_([64, BL], "h1r_t")

        # ---------- init (before x DMA overwrites row 64) ----------
        nc.vector.memset(HB[64:66, :], 1.0)  # ones rows; row 64 then x-filled
        nc.vector.memset(HB[0:64, T * BL:(T + 1) * BL], 0.0)  # hb_hat[-1] = 0
        nc.vector.memset(cA[0][:], 0.0)

        # ---------- input DMAs ----------
        nc.sync.dma_start(out=HB[64:65, :], in_=xs)
        for r in range(4):
            nc.sync.dma_start(out=xq[32 * r:32 * r + 1, :], in_=xf[r:r + 1, :])
        nc.sync.dma_start(out=l0f_t[:], in_=l0f)
        nc.sync.dma_start(out=l0r_t[:], in_=l0r)
        for r in range(4):
            nc.sync.dma_start(out=xw_t[32 * r:32 * r + 1, :], in_=xw[r:r + 1, :])
        nc.sync.dma_start(out=l1_t[:], in_=l1)
        nc.sync.dma_start(out=w1_t[:], in_=w1)
        nc.sync.dma_start(out=w1r_t[:], in_=w1r)
        nc.sync.dma_start(out=b1r_t[:], in_=b1r)
        nc.sync.dma_start(out=fcw_t[:], in_=fcw)
        nc.sync.dma_start(out=fcb_t[:], in_=fcb)

        # ---------- pools ----------
        psum = ctx.enter_context(tc.tile_pool(name="psum", bufs=3, space="PSUM"))
        gates = ctx.enter_context(tc.tile_pool(name="gates", bufs=3))
        tmp = ctx.enter_context(tc.tile_pool(name="tmp", bufs=3))

        nc.vector.memset(ones_t[:], 1.0)

        def cell(p_if, p_og, c_cur, c_nxt, h_out, width):
            """Post-matmul LSTM cell math on [.., width] tiles."""
            g_if = gates.tile([128, width], F32, tag="g_if")
            g_og = gates.tile([128, width], F32, tag="g_og")
            nc.scalar.activation(g_if[:], p_if, SIG)
            nc.scalar.activation(g_og[:], p_og, SIG)
            # gate layout: if-half = [f; i], og-half = [o; g] so every
            # tensor-tensor input pair shares a base partition (HW rule).
            # t1 = (sigmoid(2g) - 0.5) * i_tilde    ( = tanh(g)*i/2 )
            t1 = tmp.tile([64, width], F32, tag="t1")
            nc.vector.scalar_tensor_tensor(
                t1[:], g_og[64:128, :], 0.5, g_if[64:128, :], op0=SUB, op1=MULT)
            # fc_ = f_tilde * c
            fc_ = tmp.tile([64, width], F32, tag="fc")
            nc.vector.tensor_mul(fc_[:], g_if[0:64, :], c_cur)
            # c' = 2*t1 + fc_
            nc.vector.scalar_tensor_tensor(
                c_nxt, t1[:], 2.0, fc_[:], op0=MULT, op1=ADD)
            # sc = sigmoid(2c')
            sc = tmp.tile([64, width], F32, tag="sc")
            nc.scalar.activation(sc[:], c_nxt, SIG, scale=2.0)
            # h_hat = (sc - 0.5) * o_tilde   ( = o*tanh(c)/2 )
            nc.vector.scalar_tensor_tensor(
                h_out, sc[:], 0.5, g_og[0:64, :], op0=SUB, op1=MULT)

        # ---------- phase A: bwd0 scan, s = 0..T-1 ----------
        for s in range(T):
            rd = (T - s) * BL
            wr = (T - 1 - s) * BL
            rhs = HB[0:66, rd:rd + BL]
            p_if = psum.tile([128, BL], F32, tag="p_if")
            p_og = psum.tile([128, BL], F32, tag="p_og")
            nc.tensor.matmul(p_if[:], l0r_t[0:66, 0:128], rhs, start=True, stop=True)
            nc.tensor.matmul(p_og[:], l0r_t[0:66, 128:256], rhs, start=True, stop=True)
            cell(p_if[:], p_og[:], cA[s % 2][:], cA[(s + 1) % 2][:],
                 HB[0:64, wr:wr + BL], BL)

        # ---------- phase B: fwd0 (step t) + layer1 fwd (step t-1) ----------
        nc.vector.memset(sB[0][0:64, :], 0.0)
        nc.vector.memset(sB[1][0:64, BL:2 * BL], 0.0)
        nc.vector.memset(sB[0][64:65, :], 1.0)
        nc.vector.memset(sB[1][64:65, :], 1.0)
        nc.vector.memset(cB[0][:, 0:BL], 0.0)
        nc.vector.memset(cB[1][:, BL:2 * BL], 0.0)

        for t in range(T + 1):
            cur, nxt = sB[t % 2], sB[(t + 1) % 2]
            fwd_on = t < T
            l1_on = t >= 1
            a0 = 0 if fwd_on else BL
            a1 = 2 * BL if l1_on else BL
            width = a1 - a0
            p_if = psum.tile([128, 2 * BL], F32, tag="p_if")
            p_og = psum.tile([128, 2 * BL], F32, tag="p_og")
            if fwd_on:
                xr0 = 32 * (t % 4)
                xcols = xq[xr0:xr0 + 1, (t // 4) * BL:(t // 4 + 1) * BL]
                nc.tensor.matmul(p_if[:, 0:BL], xw_t[xr0:xr0 + 1, 0:128], xcols,
                                 start=True, stop=False, tile_position=(xr0, 0))
                nc.tensor.matmul(p_og[:, 0:BL], xw_t[xr0:xr0 + 1, 128:256], xcols,
                                 start=True, stop=False, tile_position=(xr0, 0))
                rhs = cur[0:65, 0:BL]
                nc.tensor.matmul(p_if[:, 0:BL], l0f_t[0:65, 0:128], rhs,
                                 start=False, stop=True)
                nc.tensor.matmul(p_og[:, 0:BL], l0f_t[0:65, 128:256], rhs,
                                 start=False, stop=True)
            if l1_on:
                # layer-1 step t-1: inputs h_hat_f[t-1] (cur fwd rows),
                # hb_hat[t-1] (HB slot t-1), own state h1_hat[t-2] + bias row.
                hf = cur[0:64, 0:BL]
                hb = HB[0:64, (t - 1) * BL:t * BL]
                s1 = cur[0:65, BL:2 * BL]
                nc.tensor.matmul(p_if[:, BL:2 * BL], w1_t[0:64, 0:128], hf,
                                 start=True, stop=False)
                nc.tensor.matmul(p_if[:, BL:2 * BL], w1_t[0:64, 256:384], hb,
                                 start=False, stop=False)
                nc.tensor.matmul(p_if[:, BL:2 * BL], l1_t[0:65, 0:128], s1,
                                 start=False, stop=True)
                nc.tensor.matmul(p_og[:, BL:2 * BL], w1_t[0:64, 128:256], hf,
                                 start=True, stop=False)
                nc.tensor.matmul(p_og[:, BL:2 * BL], w1_t[0:64, 384:512], hb,
                                 start=False, stop=False)
                nc.tensor.matmul(p_og[:, BL:2 * BL], l1_t[0:65, 128:256], s1,
                                 start=False, stop=True)
            cell(p_if[:, a0:a1], p_og[:, a0:a1],
                 cB[t % 2][:, a0:a1], cB[(t + 1) % 2][:, a0:a1],
                 nxt[0:64, a0:a1], width)
            if t == 0:
                # wipe the spurious layer-1 half so iteration 1 sees zeros
                nc.vector.memset(nxt[0:64, BL:2 * BL], 0.0)
                nc.vector.memset(cB[1][:, BL:2 * BL], 0.0)

        # ---------- phase C: one bwd1 step at t=T-1, then FC ----------
        sfin = sB[T % 2]          # h_hat_f[T-1] in [0:64, 0:BL]
        s1fin = sB[(T + 1) % 2]   # h1_hat[T-1] in [0:64, BL:2BL]
        hbT = HB[0:64, (T - 1) * BL:T * BL]
        p_if = psum.tile([128, BL], F32, tag="p_if")
        p_og = psum.tile([128, BL], F32, tag="p_og")
        nc.tensor.matmul(p_if[:], w1r_t[0:64, 0:128], sfin[0:64, 0:BL],
                         start=True, stop=False)
        nc.tensor.matmul(p_if[:], b1r_t[0:1, 0:128], ones_t[0:1, :],
                         start=False, stop=False)
        nc.tensor.matmul(p_if[:], w1r_t[0:64, 256:384], hbT,
                         start=False, stop=True)
        nc.tensor.matmul(p_og[:], w1r_t[0:64, 128:256], sfin[0:64, 0:BL],
                         start=True, stop=False)
        nc.tensor.matmul(p_og[:], b1r_t[0:1, 128:256], ones_t[0:1, :],
                         start=False, stop=False)
        nc.tensor.matmul(p_og[:], w1r_t[0:64, 384:512], hbT,
                         start=False, stop=True)
        g_if = gates.tile([128, BL], F32, tag="g_if")
        g_og = gates.tile([128, BL], F32, tag="g_og")
        nc.scalar.activation(g_if[:], p_if[:], SIG)
        nc.scalar.activation(g_og[:], p_og[:], SIG)
        t1 = tmp.tile([64, BL], F32, tag="t1")
        nc.vector.scalar_tensor_tensor(
            t1[:], g_og[64:128, :], 0.5, g_if[64:128, :], op0=SUB, op1=MULT)
        # c = 2*t1 (c_prev = 0); sc = sigmoid(2c) = sigmoid(4*t1)
        sc = tmp.tile([64, BL], F32, tag="sc")
        nc.scalar.activation(sc[:], t1[:], SIG, scale=4.0)
        nc.vector.scalar_tensor_tensor(
            h1r_t[:], sc[:], 0.5, g_og[0:64, :], op0=SUB, op1=MULT)

        p_fc = psum.tile([1, BL], F32, tag="p_fc", bufs=1)
        nc.tensor.matmul(p_fc[:], fcw_t[0:64, 0:1], s1fin[0:64, BL:2 * BL],
                         start=True, stop=False)
        nc.tensor.matmul(p_fc[:], fcw_t[0:64, 1:2], h1r_t[:],
                         start=False, stop=True)
        out_sb = T_([1, BL], "out_sb")
        nc.scalar.activation(out_sb[:], p_fc[:], IDENT, bias=fcb_t[0:1, 0:1])
        nc.sync.dma_start(out=out, in_=out_sb[:])

    nc.compile()
    return nc


def _permute_gates(m):
    """[i,f,g,o] column blocks -> [f,i,o,g] (per 256-wide block)."""
    out = np.empty_like(m)
    for b0 in range(0, m.shape[1], 256):
        blk = m[:, b0:b0 + 256]
        out[:, b0:b0 + 256] = np.concatenate(
            [blk[:, 64:128], blk[:, 0:64], blk[:, 192:256], blk[:, 128:192]],
            axis=1)
    return out


def prep_l0r(w_ih, w_hh, b_ih, b_hh):
    """bwd-layer-0 lhsT [66, 256]: rows 0:64 w_hh^T*2 (h_hat comp), row 64
    w_ih column, row 65 bias; g-gate columns doubled for sigmoid(2g)."""
    m = np.zeros((66, 256), np.float32)
    m[0:64] = w_hh.T * 2.0
    m[64] = w_ih[:, 0]
    m[65] = b_ih + b_hh
    m[:, 128:192] *= 2.0
    return _permute_gates(m)


def prep_l0f(w_ih, w_hh, b_ih, b_hh):
    """fwd-layer-0: recurrence lhsT [65, 256] (row 64 = bias) + x-weight row."""
    m = np.zeros((65, 256), np.float32)
    m[0:64] = w_hh.T * 2.0
    m[64] = b_ih + b_hh
    m[:, 128:192] *= 2.0
    xw = np.tile(w_ih[:, 0].astype(np.float32).reshape(1, 256), (4, 1))
    xw[:, 128:192] *= 2.0
    return _permute_gates(m), _permute_gates(xw)


def prep_l1(w_ih, w_hh, b_ih, b_hh):
    rec = np.zeros((65, 256), np.float32)
    rec[0:64] = w_hh.T * 2.0
    rec[64] = b_ih + b_hh
    rec[:, 128:192] *= 2.0
    wx = np.zeros((64, 512), np.float32)
    wx[:, 0:256] = w_ih.T[0:64] * 2.0      # h_f part
    wx[:, 256:512] = w_ih.T[64:128] * 2.0  # h_b part
    wx[:, 128:192] *= 2.0
    wx[:, 384:448] *= 2.0
    return _permute_gates(rec), _permute_gates(wx)


def make_inmaps(x, w_ih_l0, w_hh_l0, b_ih_l0, b_hh_l0,
                w_ih_l0r, w_hh_l0r, b_ih_l0r, b_hh_l0r,
                w_ih_l1, w_hh_l1, b_ih_l1, b_hh_l1,
                w_ih_l1r, w_hh_l1r, b_ih_l1r, b_hh_l1r,
                fc_w, fc_b, T, BL, ncores):
    l0f, xw = prep_l0f(w_ih_l0, w_hh_l0, b_ih_l0, b_hh_l0)
    l0r = prep_l0r(w_ih_l0r, w_hh_l0r, b_ih_l0r, b_hh_l0r)
    l1, w1 = prep_l1(w_ih_l1, w_hh_l1, b_ih_l1, b_hh_l1)
    l1r, w1r = prep_l1(w_ih_l1r, w_hh_l1r, b_ih_l1r, b_hh_l1r)
    b1r = l1r[64:65].copy()  # bias row incl. g-doubling, [1, 256]
    fcw = np.zeros((64, 2), np.float32)
    fcw[:, 0] = fc_w[0, 0:64] * 2.0
    fcw[:, 1] = fc_w[0, 64:128] * 2.0
    fcb = np.asarray(fc_b, np.float32).reshape(1, 1)

    xall = np.asarray(x[..., 0], np.float32)  # [B, T]
    in_maps = []
    for c in range(ncores):
        xb = xall[c * BL:(c + 1) * BL, :]       # [BL, T]
        xtm = np.ascontiguousarray(xb.T)        # [T, BL] t-major
        xslots = np.zeros((T + 1, BL), np.float32)
        xslots[1:] = xtm                        # slot j = x[j-1]
        xquart = np.ascontiguousarray(
            xtm.reshape(T // 4, 4, BL).transpose(1, 0, 2).reshape(4, -1))
        in_maps.append({
            "xs": xslots.reshape(1, -1), "xf": xquart,
            "l0f": l0f, "l0r": l0r, "xw": xw, "l1": l1,
            "w1": w1, "w1r": w1r, "b1r": b1r, "fcw": fcw, "fcb": fcb,
        })
    return in_maps


_NC_CACHE = {}


def kernel(**inputs) -> np.ndarray:
    T, BL = 512, 64
    key = (T, BL)
    if key not in _NC_CACHE:
        _NC_CACHE[key] = build_nc(T, BL)
    nc = _NC_CACHE[key]
    in_maps = make_inmaps(T=T, BL=BL, ncores=NCORES, **inputs)
    res = run_bass_kernel_spmd(nc, in_maps, core_ids=list(range(NCORES)))
    return np.concatenate([r["out"] for r in res.results], axis=0)
